# revision 31
# baseline (speedup 1.0000x reference)
"""ConvNext block kernel for Trainium2, data-parallel over batch on 8 cores.

Per-core work (2 samples): depthwise circular conv (DVE), GroupNorm stats
(accum_out + ones-matmul reduce/broadcast), FiLM folded into per-channel
affine, two pointwise matmuls in bf16 (gamma folded into pw2 weights on
host), ReLU+bias fused into PSUM eviction, residual add in fp32.
"""

import numpy as np
import ml_dtypes

import concourse.bass as bass
import concourse.tile as tile
from concourse import bacc, mybir
from concourse.bass_utils import run_bass_kernel_spmd

F32 = mybir.dt.float32
BF16 = mybir.dt.bfloat16
F8 = mybir.dt.float8e4
NP_BF16 = ml_dtypes.bfloat16
NP_F8 = ml_dtypes.float8_e4m3
W1SCALE = 16.0   # host-side prescale of fp8 pw1 weights; undone in relu scale

B, C, L = 16, 256, 4096
K, PAD = 7, 3
EMB, HID = 128, 1024
EPS = 1e-6
NCORES = 8
BL = B // NCORES          # samples per core
P = 128
NCC = C // P              # 2 channel chunks
NDC = HID // P            # 8 hidden chunks
LT = 512                  # L tile (one PSUM bank fp32)
NLT = L // LT             # 8
NLTG = NLT // 2           # 4 groups of 2 L-tiles
NTOT = float(C * L)

AO = mybir.AluOpType
AF = mybir.ActivationFunctionType


PE_TAPS = (1, 3, 5, 6)
DVE_TAPS = (2, 4)


def _emit(nc, tc, xh, outh, dr):
    """Emit the per-core program. dr: dict of weight dram handles."""
    import contextlib
    ctx = contextlib.ExitStack()
    singles = ctx.enter_context(tc.tile_pool(name="singles", bufs=1))
    small = ctx.enter_context(tc.tile_pool(name="small", bufs=1))
    xbuf = ctx.enter_context(tc.tile_pool(name="xbuf", bufs=2))
    ybuf = ctx.enter_context(tc.tile_pool(name="ybuf", bufs=2))
    y8buf = ctx.enter_context(tc.tile_pool(name="y8buf", bufs=2))
    ydbuf = ctx.enter_context(tc.tile_pool(name="ydbuf", bufs=2))
    tmpbuf = ctx.enter_context(tc.tile_pool(name="tmpbuf", bufs=2))
    sqbuf = ctx.enter_context(tc.tile_pool(name="sqbuf", bufs=1))
    hbuf = ctx.enter_context(tc.tile_pool(name="hbuf", bufs=2))
    x32buf = ctx.enter_context(tc.tile_pool(name="x32buf", bufs=3))
    obuf = ctx.enter_context(tc.tile_pool(name="obuf", bufs=3))
    ph_pool = ctx.enter_context(tc.tile_pool(name="ph", bufs=2, space="PSUM"))
    pb_pool = ctx.enter_context(tc.tile_pool(name="pb", bufs=2, space="PSUM"))

    # ---- constants / weights -------------------------------------------
    w1t8 = singles.tile([P, NCC, HID], F8)
    nc.sync.dma_start(out=w1t8, in_=dr["w1t8"][:].rearrange("(cc p) d -> p cc d", p=P))
    w2t8 = singles.tile([P, NDC, C], F8)
    nc.sync.dma_start(out=w2t8, in_=dr["w2t8"][:].rearrange("(dc p) c -> p dc c", p=P))
    geff = singles.tile([P, 1], F32)
    nc.sync.dma_start(out=geff, in_=dr["geff"][:])
    fwt = singles.tile([P, 2 * C], F32)
    nc.sync.dma_start(out=fwt, in_=dr["film_wt"][:])
    tT = singles.tile([P, BL], F32)
    nc.sync.dma_start(out=tT, in_=dr["tT"][:])
    dww = singles.tile([P, NCC, K], F32)
    nc.sync.dma_start(out=dww, in_=dr["dww"][:])
    dwd = singles.tile([P, NCC * len(PE_TAPS), P], BF16)
    nc.sync.dma_start(
        out=dwd, in_=dr["dwdiag"][:].rearrange("(g p) q -> p g q", p=P))
    dwb = singles.tile([P, NCC], F32)
    nc.sync.dma_start(out=dwb, in_=dr["dwb"][:])
    fb = singles.tile([P, 4], F32)
    nc.sync.dma_start(out=fb, in_=dr["fb"][:])
    b1 = singles.tile([P, NDC], F32)
    nc.sync.dma_start(out=b1, in_=dr["b1"][:])
    b2g = singles.tile([P, NCC], F32)
    nc.sync.dma_start(out=b2g, in_=dr["b2g"][:])
    ones = singles.tile([P, P], F32)
    nc.vector.memset(ones, 1.0)
    eps_t = singles.tile([P, 1], F32)
    nc.vector.memset(eps_t, EPS)
    zero_t = singles.tile([P, 1], F32)
    nc.vector.memset(zero_t, 0.0)

    xv = [xh[b].rearrange("(cc p) l -> p cc l", p=P) for b in range(BL)]
    ov = [outh[b].rearrange("(cc p) l -> p cc l", p=P) for b in range(BL)]

    # ---- FiLM: ss[jc*128+p, b] = film_w @ t.T + film_b ------------------
    ps_film = ph_pool.tile([P, 4, BL], F32, tag="ph")
    for jc in range(4):
        nc.tensor.matmul(
            ps_film[:, jc, :],
            lhsT=fwt[:, jc * P:(jc + 1) * P],
            rhs=tT,
            start=True, stop=True,
            skip_group_check=True,
        )
    ss = small.tile([P, 4, BL], F32, tag="ss")
    nc.vector.tensor_copy(out=ss, in_=ps_film)

    # preload the sqrt table set so no ACT table switch lands mid-kernel
    dummy1 = singles.tile([P, 1], F32)
    nc.scalar.activation(out=dummy1, in_=eps_t, func=AF.Sqrt, bias=eps_t[:, 0:1])

    # per-sample state kept across emission phases
    st = [dict() for _ in range(BL)]
    QL = 2 * LT  # 1024, conv processing quantum (= one lt pair)

    def load_x(b):
        """chunked cast-DMA (fp32->bf16) so conv can start on early chunks"""
        xb = xbuf.tile([P, NCC, L + 2 * PAD], BF16, tag="xb")
        nc.gpsimd.dma_start(out=xb[:, :, 0:PAD], in_=xv[b][:, :, L - PAD:L])
        for q in range(NLTG):
            nc.gpsimd.dma_start(
                out=xb[:, :, PAD + q * QL:PAD + (q + 1) * QL],
                in_=xv[b][:, :, q * QL:(q + 1) * QL])
        nc.gpsimd.dma_start(out=xb[:, :, PAD + L:PAD + L + PAD],
                            in_=xv[b][:, :, 0:PAD])
        st[b]["xb"] = xb

    def conv(b):
        """per (cc, quarter): DVE taps 0,2,4 -> PE taps 1,3,5,6 -> combine
        (+sum_y accum) -> square (+sum_y2 accum)"""
        xb = st[b]["xb"]
        y = ybuf.tile([P, NCC, L], BF16, tag="y")
        pyp = small.tile([P, NCC * NLTG], F32, tag=f"pyp{b}")
        pq = small.tile([P, NCC * NLTG], F32, tag=f"pq{b}")
        partials = small.tile([P, 4], F32, tag=f"partials{b}")
        for cc in range(NCC):
            for lpp in range(NLTG // 2):   # lt-pair pairs (each = 2 quarters)
                lps = (2 * lpp, 2 * lpp + 1)
                ydqs = {}
                for lp in lps:
                    q0 = lp * QL
                    ydq = ydbuf.tile([P, QL], BF16, tag=f"ydq{lp % 2}")
                    nc.vector.tensor_scalar(
                        out=ydq, in0=xb[:, cc, q0:q0 + QL],
                        scalar1=dww[:, cc, 0:1], scalar2=dwb[:, cc:cc + 1],
                        op0=AO.mult, op1=AO.add)
                    for d in DVE_TAPS:
                        tmp = tmpbuf.tile([P, QL], BF16, tag="tmp")
                        nc.vector.tensor_scalar(
                            out=tmp, in0=xb[:, cc, q0 + d:q0 + d + QL],
                            scalar1=dww[:, cc, d:d + 1], scalar2=None,
                            op0=AO.mult)
                        nc.vector.tensor_add(out=ydq, in0=ydq, in1=tmp)
                    ydqs[lp] = ydq
                # PE taps, tap-outer so each diag weight loads once per 4 MMs
                pcs = {}
                for lp in lps:
                    pcs[lp] = ph_pool.tile([P, QL], F32, tag="ph",
                                           name=f"pc_{b}_{cc}_{lp}")
                for i, d in enumerate(PE_TAPS):
                    first = True
                    for lp in lps:
                        for half in range(2):
                            base = lp * QL + half * LT
                            mm = nc.tensor.matmul(
                                pcs[lp][:, half * LT:(half + 1) * LT],
                                lhsT=dwd[:, cc * len(PE_TAPS) + i, :],
                                rhs=xb[:, cc, base + d:base + d + LT],
                                start=(i == 0), stop=(i == len(PE_TAPS) - 1),
                                skip_group_check=True)
                            if not first:
                                mm.ins.ldweights = False
                            first = False
                for lp in lps:
                    q0 = lp * QL
                    nc.vector.scalar_tensor_tensor(
                        out=y[:, cc, q0:q0 + QL], in0=pcs[lp], scalar=1.0,
                        in1=ydqs[lp], op0=AO.mult, op1=AO.add,
                        accum_out=pyp[:, cc * NLTG + lp:cc * NLTG + lp + 1])
                    sq = sqbuf.tile([P, QL], BF16, tag="sq")
                    nc.scalar.activation(
                        out=sq, in_=y[:, cc, q0:q0 + QL], func=AF.Square,
                        bias=zero_t[:, 0:1],
                        accum_out=pq[:, cc * NLTG + lp:cc * NLTG + lp + 1])
        st[b].update(y=y, pyp=pyp, pq=pq, partials=partials)

    def stats_and_yn(b):
        y, partials = st[b]["y"], st[b]["partials"]
        nc.vector.tensor_reduce(
            out=partials[:, 0:2],
            in_=st[b]["pyp"].rearrange("p (c l) -> p c l", c=NCC),
            axis=mybir.AxisListType.X, op=AO.add)
        nc.vector.tensor_reduce(
            out=partials[:, 2:4],
            in_=st[b]["pq"].rearrange("p (c l) -> p c l", c=NCC),
            axis=mybir.AxisListType.X, op=AO.add)
        ps_st = ph_pool.tile([P, 4], F32, tag="ph")
        nc.tensor.matmul(ps_st, lhsT=ones, rhs=partials,
                         start=True, stop=True, skip_group_check=True)
        sums2 = small.tile([P, 2], F32, tag=f"sums2{b}")
        nc.vector.tensor_reduce(
            out=sums2, in_=ps_st.rearrange("p (a c) -> p a c", c=2),
            axis=mybir.AxisListType.X, op=AO.add)
        musq = small.tile([P, 2], F32, tag=f"musq{b}")
        nc.vector.tensor_scalar_mul(out=musq, in0=sums2, scalar1=1.0 / NTOT)
        mu2 = small.tile([P, 1], F32, tag=f"mu2{b}")
        nc.scalar.activation(out=mu2, in_=musq[:, 0:1], func=AF.Square,
                             bias=zero_t[:, 0:1])
        va = small.tile([P, 1], F32, tag=f"va{b}")
        nc.vector.tensor_sub(out=va, in0=musq[:, 1:2], in1=mu2)
        sd = small.tile([P, 1], F32, tag=f"sd{b}")
        nc.scalar.activation(out=sd, in_=va, func=AF.Sqrt, bias=eps_t[:, 0:1])
        rstd = small.tile([P, 1], F32, tag=f"rstd{b}")
        nc.vector.reciprocal(out=rstd, in_=sd)
        for cc in range(NCC):
            t1 = small.tile([P, 1], F32, tag=f"t1_{b}{cc}")
            nc.vector.tensor_add(out=t1, in0=ss[:, cc, b:b + 1],
                                 in1=fb[:, cc:cc + 1])
            A = small.tile([P, 1], F32, tag=f"A_{b}{cc}")
            nc.vector.tensor_scalar(out=A, in0=t1, scalar1=1.0, scalar2=rstd,
                                    op0=AO.add, op1=AO.mult)
            t2 = small.tile([P, 1], F32, tag=f"t2_{b}{cc}")
            nc.vector.tensor_add(out=t2, in0=ss[:, 2 + cc, b:b + 1],
                                 in1=fb[:, 2 + cc:3 + cc])
            negB = small.tile([P, 1], F32, tag=f"negB_{b}{cc}")
            nc.vector.scalar_tensor_tensor(
                out=negB, in0=A, scalar=musq[:, 0:1], in1=t2,
                op0=AO.mult, op1=AO.subtract)
            nc.vector.tensor_scalar(
                out=y[:, cc, :], in0=y[:, cc, :], scalar1=A, scalar2=negB,
                op0=AO.mult, op1=AO.subtract)

    def cast8(b):
        """bf16 -> fp8 cast of normalized y (gpsimd, quartered for latency)"""
        y8 = y8buf.tile([P, NCC, L], F8, tag="y8")
        for q in range(NLTG):
            for cc in range(NCC):
                sl = slice(q * QL, (q + 1) * QL)
                nc.gpsimd.tensor_copy(out=y8[:, cc, sl], in_=st[b]["y"][:, cc, sl])
        st[b]["y8"] = y8

    def pw1_ltg(b, ltg):
        """fp8 DoubleRow matmul (full C contraction per MM) + relu evict"""
        y8 = st[b]["y8"]
        h = hbuf.tile([P, NDC, 2 * LT], F8, tag="h")
        for dc in range(NDC):
            ph = ph_pool.tile([P, 2 * LT], F32, tag="ph")
            for half in range(2):
                l0 = ltg * 2 * LT + half * LT
                mm = nc.tensor.matmul(
                    ph[:, half * LT:(half + 1) * LT],
                    lhsT=w1t8[:, :, dc * P:(dc + 1) * P],
                    rhs=y8[:, :, l0:l0 + LT],
                    start=True, stop=True, skip_group_check=True,
                    perf_mode=mybir.MatmulPerfMode.DoubleRow)
                if half == 1:
                    mm.ins.ldweights = False
            nc.scalar.activation(out=h[:, dc, :], in_=ph, func=AF.Relu,
                                 scale=1.0 / W1SCALE, bias=b1[:, dc:dc + 1])
        st[b][f"h{ltg}"] = h

    def pw2_ltg(b, ltg):
        h = st[b].pop(f"h{ltg}")
        l0 = ltg * 2 * LT
        x32 = x32buf.tile([P, NCC, 2 * LT], F32, tag="x32")
        nc.sync.dma_start(out=x32, in_=xv[b][:, :, l0:l0 + 2 * LT])
        o = obuf.tile([P, NCC, 2 * LT], F32, tag="o")
        for cc in range(NCC):
            pb = pb_pool.tile([P, 2 * LT], F32, tag="pb")
            for m in range(NDC // 2):    # fp8 DoubleRow: 2 hid-chunks per MM
                first = True
                for half in range(2):
                    mm = nc.tensor.matmul(
                        pb[:, half * LT:(half + 1) * LT],
                        lhsT=w2t8[:, 2 * m:2 * m + 2, cc * P:(cc + 1) * P],
                        rhs=h[:, 2 * m:2 * m + 2, half * LT:(half + 1) * LT],
                        start=(m == 0), stop=(m == NDC // 2 - 1),
                        skip_group_check=True,
                        perf_mode=mybir.MatmulPerfMode.DoubleRow)
                    if not first:
                        mm.ins.ldweights = False
                    first = False
            # o = (psum*geff + b2g) + x   (geff undoes the fp8 weight scaling)
            nc.vector.tensor_scalar(
                out=o[:, cc, :], in0=pb, scalar1=geff[:, 0:1],
                scalar2=b2g[:, cc:cc + 1], op0=AO.mult, op1=AO.add)
            nc.vector.tensor_add(out=o[:, cc, :], in0=o[:, cc, :],
                                 in1=x32[:, cc, :])
        nc.sync.dma_start(out=ov[b][:, :, l0:l0 + 2 * LT], in_=o)

    # ---- emission schedule (engine FIFO order is emission order) --------
    load_x(0)
    load_x(1)
    conv(0)
    stats_and_yn(0)
    cast8(0)
    conv(1)
    stats_and_yn(1)
    cast8(1)
    # software-pipelined pw phase: pw1(n+1) emitted before pw2(n) so the
    # PE never idles waiting for the relu evictions feeding pw2
    units = [(b, ltg) for b in range(BL) for ltg in range(NLTG)]
    pw1_ltg(*units[0])
    for i in range(1, len(units)):
        pw1_ltg(*units[i])
        pw2_ltg(*units[i - 1])
    pw2_ltg(*units[-1])
    ctx.close()


def _build():
    nc = bacc.Bacc()
    xh = nc.dram_tensor("x", [BL, C, L], F32, kind="ExternalInput")
    dr = {
        "tT": nc.dram_tensor("tT", [EMB, BL], F32, kind="ExternalInput"),
        "dww": nc.dram_tensor("dww", [P, NCC, K], F32, kind="ExternalInput"),
        "dwdiag": nc.dram_tensor("dwdiag", [NCC * len(PE_TAPS) * P, P], BF16,
                                 kind="ExternalInput"),
        "dwb": nc.dram_tensor("dwb", [P, NCC], F32, kind="ExternalInput"),
        "film_wt": nc.dram_tensor("film_wt", [EMB, 2 * C], F32, kind="ExternalInput"),
        "fb": nc.dram_tensor("fb", [P, 4], F32, kind="ExternalInput"),
        "w1t8": nc.dram_tensor("w1t8", [C, HID], F8, kind="ExternalInput"),
        "b1": nc.dram_tensor("b1", [P, NDC], F32, kind="ExternalInput"),
        "w2t8": nc.dram_tensor("w2t8", [HID, C], F8, kind="ExternalInput"),
        "geff": nc.dram_tensor("geff", [P, 1], F32, kind="ExternalInput"),
        "b2g": nc.dram_tensor("b2g", [P, NCC], F32, kind="ExternalInput"),
    }
    outh = nc.dram_tensor("out", [BL, C, L], F32, kind="ExternalOutput")
    with tile.TileContext(nc) as tc:
        _emit(nc, tc, xh, outh, dr)
    if not nc.is_finalized():
        nc.finalize()
    return nc


_NC = None


def _prep_maps(x, t, dw_w, dw_b, film_w, film_b, pw1_w, pw1_b, pw2_w, pw2_b,
               gamma):
    f32 = np.float32
    w1t8 = np.ascontiguousarray(pw1_w.T * W1SCALE).astype(NP_F8)   # [C, HID]
    # fp8 pw2 weights: gamma (~1e-6) would underflow fp8, so normalize by a
    # power of two g0 and restore via the per-partition geff scalar on device
    gmax = float(np.max(np.abs(gamma)))
    g0 = 2.0 ** np.floor(np.log2(gmax)) if gmax > 0 else 1.0
    w2t8 = np.ascontiguousarray(
        (pw2_w * (gamma / g0) * W1SCALE).T).astype(NP_F8)          # [HID, C]
    geff = np.full((P, 1), g0 / W1SCALE, dtype=f32)
    b2g = (pw2_b * gamma[:, 0]).astype(f32)
    ntap = len(PE_TAPS)
    dwdiag = np.zeros((ntap * NCC, P, P), dtype=f32)
    for cc in range(NCC):
        for ti, d in enumerate(PE_TAPS):
            np.fill_diagonal(dwdiag[cc * ntap + ti],
                             dw_w[cc * P:(cc + 1) * P, 0, d])
    shared = {
        "dww": np.ascontiguousarray(
            dw_w[:, 0, :].reshape(NCC, P, K).transpose(1, 0, 2)).astype(f32),
        "dwdiag": dwdiag.reshape(ntap * NCC * P, P).astype(NP_BF16),
        "dwb": np.ascontiguousarray(dw_b.reshape(NCC, P).T).astype(f32),
        "film_wt": np.ascontiguousarray(film_w.T).astype(f32),
        "fb": np.ascontiguousarray(film_b.reshape(4, P).T).astype(f32),
        "w1t8": w1t8,
        "b1": np.ascontiguousarray(pw1_b.reshape(NDC, P).T).astype(f32),
        "w2t8": w2t8,
        "geff": geff,
        "b2g": np.ascontiguousarray(b2g.reshape(NCC, P).T).astype(f32),
    }
    maps = []
    for i in range(NCORES):
        sl = slice(i * BL, (i + 1) * BL)
        m = dict(shared)
        m["x"] = np.ascontiguousarray(x[sl]).astype(f32)
        m["tT"] = np.ascontiguousarray(t[sl].T).astype(f32)
        maps.append(m)
    return maps


def _run(inputs, trace=False, **kw):
    global _NC
    if _NC is None:
        _NC = _build()
    maps = _prep_maps(**inputs)
    res = run_bass_kernel_spmd(_NC, maps, core_ids=list(range(NCORES)),
                               trace=trace, **kw)
    out = np.concatenate([r["out"] for r in res.results], axis=0)
    return out.astype(np.float32), res


def kernel(**inputs):
    out, _ = _run(inputs, trace=False)
    return out


# revision 37
# speedup vs baseline: 1.0421x; 1.0421x over previous
"""ConvNext block kernel for Trainium2, data-parallel over batch on 8 cores.

Per-core work (2 samples): depthwise circular conv (DVE), GroupNorm stats
(accum_out + ones-matmul reduce/broadcast), FiLM folded into per-channel
affine, two pointwise matmuls in bf16 (gamma folded into pw2 weights on
host), ReLU+bias fused into PSUM eviction, residual add in fp32.
"""

import numpy as np
import ml_dtypes

import concourse.bass as bass
import concourse.tile as tile
from concourse import bacc, mybir
from concourse.bass_utils import run_bass_kernel_spmd

F32 = mybir.dt.float32
BF16 = mybir.dt.bfloat16
F8 = mybir.dt.float8e4
NP_BF16 = ml_dtypes.bfloat16
NP_F8 = ml_dtypes.float8_e4m3
W1SCALE = 16.0   # host-side prescale of fp8 pw1 weights; undone in relu scale

B, C, L = 16, 256, 4096
K, PAD = 7, 3
EMB, HID = 128, 1024
EPS = 1e-6
NCORES = 8
BL = B // NCORES          # samples per core
P = 128
NCC = C // P              # 2 channel chunks
NDC = HID // P            # 8 hidden chunks
LT = 512                  # L tile (one PSUM bank fp32)
NLT = L // LT             # 8
NLTG = NLT // 2           # 4 groups of 2 L-tiles
NTOT = float(C * L)

AO = mybir.AluOpType
AF = mybir.ActivationFunctionType


PE_TAPS = (1, 3, 5, 6)
DVE_TAPS = (2, 4)


def _emit(nc, tc, xh, outh, dr):
    """Emit the per-core program. dr: dict of weight dram handles."""
    import contextlib
    ctx = contextlib.ExitStack()
    singles = ctx.enter_context(tc.tile_pool(name="singles", bufs=1))
    small = ctx.enter_context(tc.tile_pool(name="small", bufs=1))
    xbuf = ctx.enter_context(tc.tile_pool(name="xbuf", bufs=2))
    ybuf = ctx.enter_context(tc.tile_pool(name="ybuf", bufs=2))
    y8buf = ctx.enter_context(tc.tile_pool(name="y8buf", bufs=2))
    ydbuf = ctx.enter_context(tc.tile_pool(name="ydbuf", bufs=2))
    tmpbuf = ctx.enter_context(tc.tile_pool(name="tmpbuf", bufs=2))
    sqbuf = ctx.enter_context(tc.tile_pool(name="sqbuf", bufs=1))
    hbuf = ctx.enter_context(tc.tile_pool(name="hbuf", bufs=2))
    obuf = ctx.enter_context(tc.tile_pool(name="obuf", bufs=3))
    ph_pool = ctx.enter_context(tc.tile_pool(name="ph", bufs=2, space="PSUM"))
    pb_pool = ctx.enter_context(tc.tile_pool(name="pb", bufs=2, space="PSUM"))

    # ---- constants / weights -------------------------------------------
    w1t8 = singles.tile([P, NCC, HID], F8)
    nc.sync.dma_start(out=w1t8, in_=dr["w1t8"][:].rearrange("(cc p) d -> p cc d", p=P))
    w2t8 = singles.tile([P, NDC, C], F8)
    nc.sync.dma_start(out=w2t8, in_=dr["w2t8"][:].rearrange("(dc p) c -> p dc c", p=P))
    geff = singles.tile([P, 1], F32)
    nc.sync.dma_start(out=geff, in_=dr["geff"][:])
    fwt = singles.tile([P, 2 * C], F32)
    nc.sync.dma_start(out=fwt, in_=dr["film_wt"][:])
    tT = singles.tile([P, BL], F32)
    nc.sync.dma_start(out=tT, in_=dr["tT"][:])
    dww = singles.tile([P, NCC, K], F32)
    nc.sync.dma_start(out=dww, in_=dr["dww"][:])
    dwd = singles.tile([P, NCC * len(PE_TAPS), P], BF16)
    nc.sync.dma_start(
        out=dwd, in_=dr["dwdiag"][:].rearrange("(g p) q -> p g q", p=P))
    dwb = singles.tile([P, NCC], F32)
    nc.sync.dma_start(out=dwb, in_=dr["dwb"][:])
    fb = singles.tile([P, 4], F32)
    nc.sync.dma_start(out=fb, in_=dr["fb"][:])
    b1 = singles.tile([P, NDC], F32)
    nc.sync.dma_start(out=b1, in_=dr["b1"][:])
    b2g = singles.tile([P, NCC], F32)
    nc.sync.dma_start(out=b2g, in_=dr["b2g"][:])
    ones = singles.tile([P, P], F32)
    nc.vector.memset(ones, 1.0)
    eps_t = singles.tile([P, 1], F32)
    nc.vector.memset(eps_t, EPS)
    zero_t = singles.tile([P, 1], F32)
    nc.vector.memset(zero_t, 0.0)

    xv = [xh[b].rearrange("(cc p) l -> p cc l", p=P) for b in range(BL)]
    ov = [outh[b].rearrange("(cc p) l -> p cc l", p=P) for b in range(BL)]

    # ---- FiLM: ss[jc*128+p, b] = film_w @ t.T + film_b ------------------
    ps_film = ph_pool.tile([P, 4, BL], F32, tag="ph")
    for jc in range(4):
        nc.tensor.matmul(
            ps_film[:, jc, :],
            lhsT=fwt[:, jc * P:(jc + 1) * P],
            rhs=tT,
            start=True, stop=True,
            skip_group_check=True,
        )
    ss = small.tile([P, 4, BL], F32, tag="ss")
    nc.vector.tensor_copy(out=ss, in_=ps_film)

    # preload the sqrt table set so no ACT table switch lands mid-kernel
    dummy1 = singles.tile([P, 1], F32)
    nc.scalar.activation(out=dummy1, in_=eps_t, func=AF.Sqrt, bias=eps_t[:, 0:1])

    # per-sample state kept across emission phases
    st = [dict() for _ in range(BL)]
    QL = 2 * LT  # 1024, conv processing quantum (= one lt pair)

    def load_x(b):
        """chunked cast-DMA (fp32->bf16) so conv can start on early chunks"""
        xb = xbuf.tile([P, NCC, L + 2 * PAD], BF16, tag="xb")
        nc.gpsimd.dma_start(out=xb[:, :, 0:PAD], in_=xv[b][:, :, L - PAD:L])
        for q in range(NLTG):
            nc.gpsimd.dma_start(
                out=xb[:, :, PAD + q * QL:PAD + (q + 1) * QL],
                in_=xv[b][:, :, q * QL:(q + 1) * QL])
        nc.gpsimd.dma_start(out=xb[:, :, PAD + L:PAD + L + PAD],
                            in_=xv[b][:, :, 0:PAD])
        st[b]["xb"] = xb

    def conv(b):
        """per (cc, quarter): DVE taps 0,2,4 -> PE taps 1,3,5,6 -> combine
        (+sum_y accum) -> square (+sum_y2 accum)"""
        xb = st[b]["xb"]
        y = ybuf.tile([P, NCC, L], BF16, tag="y")
        pyp = small.tile([P, NCC * NLTG], F32, tag=f"pyp{b}")
        pq = small.tile([P, NCC * NLTG], F32, tag=f"pq{b}")
        partials = small.tile([P, 4], F32, tag=f"partials{b}")
        for cc in range(NCC):
            for lpp in range(NLTG // 2):   # lt-pair pairs (each = 2 quarters)
                lps = (2 * lpp, 2 * lpp + 1)
                ydqs = {}
                for lp in lps:
                    q0 = lp * QL
                    ydq = ydbuf.tile([P, QL], BF16, tag=f"ydq{lp % 2}")
                    nc.vector.tensor_scalar(
                        out=ydq, in0=xb[:, cc, q0:q0 + QL],
                        scalar1=dww[:, cc, 0:1], scalar2=dwb[:, cc:cc + 1],
                        op0=AO.mult, op1=AO.add)
                    for d in DVE_TAPS:
                        tmp = tmpbuf.tile([P, QL], BF16, tag="tmp")
                        nc.vector.tensor_scalar(
                            out=tmp, in0=xb[:, cc, q0 + d:q0 + d + QL],
                            scalar1=dww[:, cc, d:d + 1], scalar2=None,
                            op0=AO.mult)
                        nc.vector.tensor_add(out=ydq, in0=ydq, in1=tmp)
                    ydqs[lp] = ydq
                # PE taps, tap-outer so each diag weight loads once per 4 MMs
                pcs = {}
                for lp in lps:
                    pcs[lp] = ph_pool.tile([P, QL], F32, tag="ph",
                                           name=f"pc_{b}_{cc}_{lp}")
                for i, d in enumerate(PE_TAPS):
                    first = True
                    for lp in lps:
                        for half in range(2):
                            base = lp * QL + half * LT
                            mm = nc.tensor.matmul(
                                pcs[lp][:, half * LT:(half + 1) * LT],
                                lhsT=dwd[:, cc * len(PE_TAPS) + i, :],
                                rhs=xb[:, cc, base + d:base + d + LT],
                                start=(i == 0), stop=(i == len(PE_TAPS) - 1),
                                skip_group_check=True)
                            if not first:
                                mm.ins.ldweights = False
                            first = False
                for lp in lps:
                    q0 = lp * QL
                    nc.vector.scalar_tensor_tensor(
                        out=y[:, cc, q0:q0 + QL], in0=pcs[lp], scalar=1.0,
                        in1=ydqs[lp], op0=AO.mult, op1=AO.add,
                        accum_out=pyp[:, cc * NLTG + lp:cc * NLTG + lp + 1])
                    sq = sqbuf.tile([P, QL], BF16, tag="sq")
                    nc.scalar.activation(
                        out=sq, in_=y[:, cc, q0:q0 + QL], func=AF.Square,
                        bias=zero_t[:, 0:1],
                        accum_out=pq[:, cc * NLTG + lp:cc * NLTG + lp + 1])
        st[b].update(y=y, pyp=pyp, pq=pq, partials=partials)

    def stats_and_yn(b):
        y, partials = st[b]["y"], st[b]["partials"]
        nc.vector.tensor_reduce(
            out=partials[:, 0:2],
            in_=st[b]["pyp"].rearrange("p (c l) -> p c l", c=NCC),
            axis=mybir.AxisListType.X, op=AO.add)
        nc.vector.tensor_reduce(
            out=partials[:, 2:4],
            in_=st[b]["pq"].rearrange("p (c l) -> p c l", c=NCC),
            axis=mybir.AxisListType.X, op=AO.add)
        ps_st = ph_pool.tile([P, 4], F32, tag="ph")
        nc.tensor.matmul(ps_st, lhsT=ones, rhs=partials,
                         start=True, stop=True, skip_group_check=True)
        sums2 = small.tile([P, 2], F32, tag=f"sums2{b}")
        nc.vector.tensor_reduce(
            out=sums2, in_=ps_st.rearrange("p (a c) -> p a c", c=2),
            axis=mybir.AxisListType.X, op=AO.add)
        musq = small.tile([P, 2], F32, tag=f"musq{b}")
        # col0 = -mu (negated so B = t2 + A*(-mu) fits one STT), col1 = E[y^2]
        nc.vector.tensor_scalar_mul(out=musq[:, 0:1], in0=sums2[:, 0:1],
                                    scalar1=-1.0 / NTOT)
        nc.vector.tensor_scalar_mul(out=musq[:, 1:2], in0=sums2[:, 1:2],
                                    scalar1=1.0 / NTOT)
        mu2 = small.tile([P, 1], F32, tag=f"mu2{b}")
        nc.scalar.activation(out=mu2, in_=musq[:, 0:1], func=AF.Square,
                             bias=zero_t[:, 0:1])
        va = small.tile([P, 1], F32, tag=f"va{b}")
        nc.vector.tensor_sub(out=va, in0=musq[:, 1:2], in1=mu2)
        sd = small.tile([P, 1], F32, tag=f"sd{b}")
        nc.scalar.activation(out=sd, in_=va, func=AF.Sqrt, bias=eps_t[:, 0:1])
        rstd = small.tile([P, 1], F32, tag=f"rstd{b}")
        nc.vector.reciprocal(out=rstd, in_=sd)
        y8 = y8buf.tile([P, NCC, L], F8, tag="y8")
        for cc in range(NCC):
            t1 = small.tile([P, 1], F32, tag=f"t1_{b}{cc}")
            nc.vector.tensor_add(out=t1, in0=ss[:, cc, b:b + 1],
                                 in1=fb[:, cc:cc + 1])
            A = small.tile([P, 1], F32, tag=f"A_{b}{cc}")
            nc.vector.tensor_scalar(out=A, in0=t1, scalar1=1.0, scalar2=rstd,
                                    op0=AO.add, op1=AO.mult)
            t2 = small.tile([P, 1], F32, tag=f"t2_{b}{cc}")
            nc.vector.tensor_add(out=t2, in0=ss[:, 2 + cc, b:b + 1],
                                 in1=fb[:, 2 + cc:3 + cc])
            Bc = small.tile([P, 1], F32, tag=f"Bc_{b}{cc}")
            nc.vector.scalar_tensor_tensor(
                out=Bc, in0=A, scalar=musq[:, 0:1], in1=t2,
                op0=AO.mult, op1=AO.add)
            # yn + fp8 cast fused on ACT: y8 = Identity(y*A + B)
            for q in range(NLTG):
                sl = slice(q * QL, (q + 1) * QL)
                nc.scalar.activation(out=y8[:, cc, sl], in_=y[:, cc, sl],
                                     func=AF.Identity, scale=A, bias=Bc)
        st[b]["y8"] = y8



    def pw1_ltg(b, ltg):
        """fp8 DoubleRow matmul (full C contraction per MM) + relu evict"""
        y8 = st[b]["y8"]
        h = hbuf.tile([P, NDC, 2 * LT], F8, tag="h")
        for dc in range(NDC):
            ph = ph_pool.tile([P, 2 * LT], F32, tag="ph")
            for half in range(2):
                l0 = ltg * 2 * LT + half * LT
                mm = nc.tensor.matmul(
                    ph[:, half * LT:(half + 1) * LT],
                    lhsT=w1t8[:, :, dc * P:(dc + 1) * P],
                    rhs=y8[:, :, l0:l0 + LT],
                    start=True, stop=True, skip_group_check=True,
                    perf_mode=mybir.MatmulPerfMode.DoubleRow)
                if half == 1:
                    mm.ins.ldweights = False
            nc.scalar.activation(out=h[:, dc, :], in_=ph, func=AF.Relu,
                                 scale=1.0 / W1SCALE, bias=b1[:, dc:dc + 1])
        st[b][f"h{ltg}"] = h

    def pw2_ltg(b, ltg):
        h = st[b].pop(f"h{ltg}")
        l0 = ltg * 2 * LT
        o = obuf.tile([P, NCC, 2 * LT], F32, tag="o")
        for cc in range(NCC):
            pb = pb_pool.tile([P, 2 * LT], F32, tag="pb")
            for m in range(NDC // 2):    # fp8 DoubleRow: 2 hid-chunks per MM
                first = True
                for half in range(2):
                    mm = nc.tensor.matmul(
                        pb[:, half * LT:(half + 1) * LT],
                        lhsT=w2t8[:, 2 * m:2 * m + 2, cc * P:(cc + 1) * P],
                        rhs=h[:, 2 * m:2 * m + 2, half * LT:(half + 1) * LT],
                        start=(m == 0), stop=(m == NDC // 2 - 1),
                        skip_group_check=True,
                        perf_mode=mybir.MatmulPerfMode.DoubleRow)
                    if not first:
                        mm.ins.ldweights = False
                    first = False
            # o = psum*geff + b2g   (geff undoes the fp8 weight scaling)
            nc.vector.tensor_scalar(
                out=o[:, cc, :], in0=pb, scalar1=geff[:, 0:1],
                scalar2=b2g[:, cc:cc + 1], op0=AO.mult, op1=AO.add)
        # residual: accumulate x into o during the DMA read (SWDGE CCE add)
        nc.gpsimd.dma_start(out=o, in_=xv[b][:, :, l0:l0 + 2 * LT],
                            accum_op=AO.add)
        nc.sync.dma_start(out=ov[b][:, :, l0:l0 + 2 * LT], in_=o)

    # ---- emission schedule (engine FIFO order is emission order) --------
    load_x(0)
    load_x(1)
    conv(0)
    stats_and_yn(0)
    conv(1)
    stats_and_yn(1)
    # software-pipelined pw phase: pw1(n+1) emitted before pw2(n) so the
    # PE never idles waiting for the relu evictions feeding pw2
    units = [(b, ltg) for b in range(BL) for ltg in range(NLTG)]
    pw1_ltg(*units[0])
    for i in range(1, len(units)):
        pw1_ltg(*units[i])
        pw2_ltg(*units[i - 1])
    pw2_ltg(*units[-1])
    ctx.close()


def _build():
    nc = bacc.Bacc()
    xh = nc.dram_tensor("x", [BL, C, L], F32, kind="ExternalInput")
    dr = {
        "tT": nc.dram_tensor("tT", [EMB, BL], F32, kind="ExternalInput"),
        "dww": nc.dram_tensor("dww", [P, NCC, K], F32, kind="ExternalInput"),
        "dwdiag": nc.dram_tensor("dwdiag", [NCC * len(PE_TAPS) * P, P], BF16,
                                 kind="ExternalInput"),
        "dwb": nc.dram_tensor("dwb", [P, NCC], F32, kind="ExternalInput"),
        "film_wt": nc.dram_tensor("film_wt", [EMB, 2 * C], F32, kind="ExternalInput"),
        "fb": nc.dram_tensor("fb", [P, 4], F32, kind="ExternalInput"),
        "w1t8": nc.dram_tensor("w1t8", [C, HID], F8, kind="ExternalInput"),
        "b1": nc.dram_tensor("b1", [P, NDC], F32, kind="ExternalInput"),
        "w2t8": nc.dram_tensor("w2t8", [HID, C], F8, kind="ExternalInput"),
        "geff": nc.dram_tensor("geff", [P, 1], F32, kind="ExternalInput"),
        "b2g": nc.dram_tensor("b2g", [P, NCC], F32, kind="ExternalInput"),
    }
    outh = nc.dram_tensor("out", [BL, C, L], F32, kind="ExternalOutput")
    with tile.TileContext(nc) as tc:
        _emit(nc, tc, xh, outh, dr)
    if not nc.is_finalized():
        nc.finalize()
    return nc


_NC = None


def _prep_maps(x, t, dw_w, dw_b, film_w, film_b, pw1_w, pw1_b, pw2_w, pw2_b,
               gamma):
    f32 = np.float32
    w1t8 = np.ascontiguousarray(pw1_w.T * W1SCALE).astype(NP_F8)   # [C, HID]
    # fp8 pw2 weights: gamma (~1e-6) would underflow fp8, so normalize by a
    # power of two g0 and restore via the per-partition geff scalar on device
    gmax = float(np.max(np.abs(gamma)))
    g0 = 2.0 ** np.floor(np.log2(gmax)) if gmax > 0 else 1.0
    w2t8 = np.ascontiguousarray(
        (pw2_w * (gamma / g0) * W1SCALE).T).astype(NP_F8)          # [HID, C]
    geff = np.full((P, 1), g0 / W1SCALE, dtype=f32)
    b2g = (pw2_b * gamma[:, 0]).astype(f32)
    ntap = len(PE_TAPS)
    dwdiag = np.zeros((ntap * NCC, P, P), dtype=f32)
    for cc in range(NCC):
        for ti, d in enumerate(PE_TAPS):
            np.fill_diagonal(dwdiag[cc * ntap + ti],
                             dw_w[cc * P:(cc + 1) * P, 0, d])
    shared = {
        "dww": np.ascontiguousarray(
            dw_w[:, 0, :].reshape(NCC, P, K).transpose(1, 0, 2)).astype(f32),
        "dwdiag": dwdiag.reshape(ntap * NCC * P, P).astype(NP_BF16),
        "dwb": np.ascontiguousarray(dw_b.reshape(NCC, P).T).astype(f32),
        "film_wt": np.ascontiguousarray(film_w.T).astype(f32),
        "fb": np.ascontiguousarray(film_b.reshape(4, P).T).astype(f32),
        "w1t8": w1t8,
        "b1": np.ascontiguousarray(pw1_b.reshape(NDC, P).T).astype(f32),
        "w2t8": w2t8,
        "geff": geff,
        "b2g": np.ascontiguousarray(b2g.reshape(NCC, P).T).astype(f32),
    }
    maps = []
    for i in range(NCORES):
        sl = slice(i * BL, (i + 1) * BL)
        m = dict(shared)
        m["x"] = np.ascontiguousarray(x[sl]).astype(f32)
        m["tT"] = np.ascontiguousarray(t[sl].T).astype(f32)
        maps.append(m)
    return maps


def _run(inputs, trace=False, **kw):
    global _NC
    if _NC is None:
        _NC = _build()
    maps = _prep_maps(**inputs)
    res = run_bass_kernel_spmd(_NC, maps, core_ids=list(range(NCORES)),
                               trace=trace, **kw)
    out = np.concatenate([r["out"] for r in res.results], axis=0)
    return out.astype(np.float32), res


def kernel(**inputs):
    out, _ = _run(inputs, trace=False)
    return out
